# revision 4
# baseline (speedup 1.0000x reference)
"""Causal single-head attention (B=4, S=2048, D=1024, fp32) on 8 TRN2 NeuronCores.

Sharding: core c <-> (batch c//2, parity r=c%2). Each core owns the 8 even/odd
128-row query tiles of its batch (balanced causal work).

v2 layout (vs baseline): all on-chip operands bf16 (fp32 PSUM accumulation),
scores computed TRANSPOSED (sT[k,q] = sum_e K^T[e,k]^T Q^T[e,q]) so the exp'd
probabilities feed the P^T V matmul directly as stationary blocks - no PE
transposes, no DVE copies on the critical path. Attention runs in two slot-quad
groups (slots 0-3 over k-tiles 0-7, slots 4-7 over k-tiles 0-15); for each
k-tile only the eligible suffix of slots is computed (zero wasted FLOPs).
Softmax denominators accumulate via ones-vector matmuls into a [1, 512] PSUM
row per group, transposed to per-partition scalars with tiny PE transposes.
Whole problem fits SBUF in bf16: no DRAM partial-O roundtrip.

USE_CC=True additionally halves K/V projection work: each core projects only
its parity's 1024 keys and the batch pair exchanges halves with an AllGather
over replica groups [[0,1],[2,3],[4,5],[6,7]].

Self-contained: hardcodes shapes; reads nothing from disk.
"""
import sys

import numpy as np
import ml_dtypes

try:
    from concourse import bass, bacc, tile
except ImportError:  # concourse ships with the container, not this file
    for _p in ("/opt/trn_rl_repo", "/root/.axon_site/_ro/trn_rl_repo"):
        if _p not in sys.path:
            sys.path.append(_p)
    from concourse import bass, bacc, tile
from concourse import mybir
from concourse.bass_utils import run_bass_kernel_spmd

dt = mybir.dt
AF = mybir.ActivationFunctionType

B, S, D = 4, 2048, 1024
P = 128
ND = D // P          # 8 d-tiles (contraction of projections)
NE = D // P          # 8 e-tiles (output feature tiles)
SLOTS = 8            # q-tiles per core
NT = S // P          # 16 k-tiles per batch
NCORES = 8
SCALE = 1.0 / float(np.sqrt(D))
NEG = -1.0e30
USE_CC = False
CC_GROUPS = [[0, 1], [2, 3], [4, 5], [6, 7]]
BF = dt.bfloat16


def _emit_body(nc, tc, pools, aps):
    (sb_const, sb_xq, sb_xt, sb_qt, sb_kt, sb_vt, sb_w, sb_wv, sb_pt,
     sb_small, sb_ob, dr_cc, ps) = pools
    xqT, xT, Wq, Wk, Wv, maskT, ones_in, O = aps

    # first weight block ahead of the bulk loads so the PE can start early;
    # constants go on the Activation queue (not needed until attention)
    wb0 = sb_w.tile([P, D], BF, tag="wblk", name="wb0")
    nc.sync.dma_start(wb0[:], (Wk if USE_CC else Wq)[0:P, :])
    ones_sb = sb_const.tile([P, 1], BF, tag="ones", name="ones_sb")
    nc.scalar.dma_start(ones_sb[:], ones_in[:])
    mask_sb = sb_const.tile([P, 2 * P], dt.float32, tag="mask", name="mask_sb")
    nc.scalar.dma_start(mask_sb[:], maskT[:])
    id1 = sb_const.tile([1, 1], dt.float32, tag="id1", name="id1")
    nc.vector.memset(id1[:], 1.0)

    # ---- input DMAs (query columns on SP; key columns on the idle
    # Activation HWDGE queue so DGE configs issue in parallel)
    xqt = [sb_xq.tile([P, SLOTS * P], BF, tag=f"xq{d}", name=f"xqt{d}")
           for d in range(ND)]
    nkt_own = NT // 2 if USE_CC else NT        # k-tiles this core projects
    xt = [sb_xt.tile([P, nkt_own * P], BF, tag=f"xt{d}", name=f"xt{d}")
          for d in range(ND)]
    if USE_CC:
        for d in range(ND):
            nc.sync.dma_start(xt[d][:], xT[d * P:(d + 1) * P, :])
        for d in range(ND):
            nc.scalar.dma_start(xqt[d][:], xqT[d * P:(d + 1) * P, :])
    else:
        for d in range(ND):
            nc.sync.dma_start(xqt[d][:, 0:512], xqT[d * P:(d + 1) * P, 0:512])
            nc.scalar.dma_start(xqt[d][:, 512:1024], xqT[d * P:(d + 1) * P, 512:1024])
        for d in range(ND):
            nc.scalar.dma_start(xt[d][:], xT[d * P:(d + 1) * P, :])

    qt = [sb_qt.tile([P, SLOTS * P], BF, tag=f"qt{e}", name=f"qt{e}")
          for e in range(NE)]
    kt = [sb_kt.tile([P, NT * P], BF, tag=f"kt{e}", name=f"kt{e}")
          for e in range(NE)]
    vt = [sb_vt.tile([P, D], BF, tag=f"vt{t}", name=f"vt{t}")
          for t in range(NT)]

    def q_proj(wq_first):
        for e in range(NE):
            if e == 0 and wq_first is not None:
                wb = wq_first
            else:
                wb = sb_w.tile([P, D], BF, tag="wblk", name="wb")
                nc.sync.dma_start(wb[:], Wq[e * P:(e + 1) * P, :])
            qps = [ps.tile([P, 512], dt.float32, tag="o", bufs=4, name="qp")
                   for _ in range(2)]
            for d in range(ND):
                for c in range(2):
                    nc.tensor.matmul(qps[c][:], wb[:, d * P:(d + 1) * P],
                                     xqt[d][:, c * 512:(c + 1) * 512],
                                     start=(d == 0), stop=(d == ND - 1))
            for c in range(2):
                nc.vector.tensor_copy(qt[e][:, c * 512:(c + 1) * 512], qps[c][:])

    def k_proj(kdst, wk_first):
        nchunks = (nkt_own * P) // 512
        for e in range(NE):
            if e == 0 and wk_first is not None:
                wb = wk_first
            else:
                wb = sb_w.tile([P, D], BF, tag="wblk", name="wb")
                nc.sync.dma_start(wb[:], Wk[e * P:(e + 1) * P, :])
            kps = [ps.tile([P, 512], dt.float32, tag="o", bufs=4, name="kp")
                   for _ in range(nchunks)]
            for d in range(ND):
                for c in range(nchunks):
                    nc.tensor.matmul(kps[c][:], wb[:, d * P:(d + 1) * P],
                                     xt[d][:, c * 512:(c + 1) * 512],
                                     start=(d == 0), stop=(d == ND - 1))
            for c in range(nchunks):
                nc.vector.tensor_copy(kdst[e][:, c * 512:(c + 1) * 512], kps[c][:])

    def v_proj(vdst):
        wvs = []
        for ec in range(2):
            wv = sb_wv.tile([P, ND * 512], BF, tag="wv", name="wv")
            nc.scalar.dma_start(wv[:], Wv[ec * P:(ec + 1) * P, :])
            wvs.append(wv)
        for st in range(nkt_own):
            vps = [ps.tile([P, 512], dt.float32, tag="o", bufs=4, name="vp")
                   for _ in range(2)]
            for d in range(ND):
                for ec in range(2):
                    nc.tensor.matmul(vps[ec][:], xt[d][:, st * P:(st + 1) * P],
                                     wvs[ec][:, d * 512:(d + 1) * 512],
                                     start=(d == 0), stop=(d == ND - 1))
            for ec in range(2):
                nc.vector.tensor_copy(vdst[st][:, ec * 512:(ec + 1) * 512], vps[ec][:])

    if not USE_CC:
        q_proj(wb0)
        k_proj(kt, None)
        v_proj(vt)
    else:
        # K proj first -> AllGather K early; V proj + Q proj cover the
        # collective latency; attention scores gate only on AG-K.
        half = nkt_own * P
        kown = [sb_w.tile([P, half], BF, tag=f"ko{e}", bufs=1, name=f"kown{e}")
                for e in range(NE)]
        vown = [sb_w.tile([P, D], BF, tag=f"vo{t}", bufs=1, name=f"vown{t}")
                for t in range(nkt_own)]
        k_proj(kown, wb0)
        cck_in = dr_cc.tile([NE * P, D], BF, tag="ccki", name="cck_in")
        for e in range(NE):
            nc.sync.dma_start(cck_in[e * P:(e + 1) * P, :], kown[e][:])
        cck_out = dr_cc.tile([2 * NE * P, D], BF, tag="ccko", name="cck_out")
        nc.gpsimd.collective_compute(
            "AllGather", mybir.AluOpType.bypass, CC_GROUPS,
            ins=[cck_in[:]], outs=[cck_out[:]])
        v_proj(vown)
        ccv_in = dr_cc.tile([nkt_own * P, D], BF, tag="ccvi", name="ccv_in")
        for st in range(nkt_own):
            nc.sync.dma_start(ccv_in[st * P:(st + 1) * P, :], vown[st][:])
        ccv_out = dr_cc.tile([2 * nkt_own * P, D], BF, tag="ccvo", name="ccv_out")
        nc.gpsimd.collective_compute(
            "AllGather", mybir.AluOpType.bypass, CC_GROUPS,
            ins=[ccv_in[:]], outs=[ccv_out[:]])
        for e in range(NE):
            nc.sync.dma_start(kt[e][:, 0:half], cck_out[e * P:(e + 1) * P, :])
            nc.sync.dma_start(kt[e][:, half:NT * P],
                              cck_out[(NE + e) * P:(NE + e + 1) * P, :])
        q_proj(None)
        for t in range(NT):
            r_src, st = t // nkt_own, t % nkt_own
            nc.sync.dma_start(
                vt[t][:],
                ccv_out[(r_src * nkt_own + st) * P:(r_src * nkt_own + st + 1) * P, :])

    # ---- attention in two slot-quad groups
    # group g: slots jb..jb+3 (jb = 4g), k-tiles 0..8(g+1)-1.
    # For k-tile t only the eligible slot suffix jmin(t)=max(jb, t//2)..jb+3 is
    # computed; the first block of the suffix is the diagonal slot and gets the
    # parity mask block (t mod 2) when t//2 >= jb.
    for g in range(2):
        jb = 4 * g
        ntile = 8 * (g + 1)
        # scores + exp + denominator accumulation
        pts = {}
        den_ps = ps.tile([1, 512], dt.float32, tag="den", bufs=1, name="den_ps")
        for t in range(ntile):
            jmin = max(jb, t // 2)
            w = (jb + 4 - jmin) * P
            sps = ps.tile([P, w], dt.float32, tag="s", bufs=3, name="sps")
            for e in range(NE):
                nc.tensor.matmul(sps[:], kt[e][:, t * P:(t + 1) * P],
                                 qt[e][:, jmin * P:(jb + 4) * P],
                                 start=(e == 0), stop=(e == NE - 1))
            if t // 2 >= jb:
                mb = t % 2
                nc.vector.tensor_add(sps[:, 0:P], sps[:, 0:P],
                                     mask_sb[:, mb * P:(mb + 1) * P])
            ptile = sb_pt.tile([P, w], BF, tag=f"pt{t}", name=f"pt{t}")
            nc.scalar.activation(ptile[:], sps[:], AF.Exp, scale=SCALE)
            pts[t] = (ptile, jmin)
            if t > 0:
                pm, pjmin = pts[t - 1]
                nc.tensor.matmul(den_ps[0:1, (pjmin - jb) * P:512], ones_sb[:],
                                 pm[:], start=(t - 1 == 0), stop=False)
            if t == ntile - 1:
                off = (jmin - jb) * P
                nc.tensor.matmul(den_ps[0:1, off:512], ones_sb[:], ptile[:],
                                 start=False, stop=True)
        # denominator -> per-partition reciprocals
        den_sb = sb_small.tile([1, 512], dt.float32, tag="den_sb", name="den_sb")
        nc.vector.tensor_copy(den_sb[:], den_ps[0:1, :])
        rcp_sb = sb_small.tile([1, 512], dt.float32, tag="rcp_sb", name="rcp_sb")
        nc.vector.reciprocal(rcp_sb[:], den_sb[:])
        rec_ps = ps.tile([P, 4], dt.float32, tag="den", bufs=1, name="rec_ps")
        for jj in range(4):
            nc.tensor.transpose(rec_ps[:, jj:jj + 1],
                                rcp_sb[0:1, jj * P:(jj + 1) * P],
                                id1[:])
        rec_sb = sb_small.tile([P, 4], dt.float32, tag="rec_sb", name="rec_sb")
        nc.vector.tensor_copy(rec_sb[:], rec_ps[:])

        # P^T V with per-slot PSUM accumulation, then normalize + store
        for j in range(jb, jb + 4):
            next_t = 2 * j + 2
            opss = [ps.tile([P, 512], dt.float32, tag="o", bufs=4, name="ops")
                    for _ in range(2)]
            for t in range(next_t):
                ptile, jmin = pts[t]
                cb = (j - jmin) * P
                for h in range(2):
                    nc.tensor.matmul(opss[h][:], ptile[:, cb:cb + P],
                                     vt[t][:, h * 512:(h + 1) * 512],
                                     start=(t == 0), stop=(t == next_t - 1))
            for h in range(2):
                ob = sb_ob.tile([P, 512], dt.float32, tag="ob", name="ob")
                nc.scalar.mul(ob[:], opss[h][:], rec_sb[:, j - jb:j - jb + 1])
                nc.sync.dma_start(O[j * P:(j + 1) * P, h * 512:(h + 1) * 512], ob[:])


def build_program(reps: int = 1):
    nc = bacc.Bacc("TRN2", target_bir_lowering=False, debug=False,
                   num_devices=NCORES)

    nkt_own = NT // 2 if USE_CC else NT
    xqT_t = nc.dram_tensor("xqT", [D, SLOTS * P], BF, kind="ExternalInput")
    xT_t = nc.dram_tensor("xT", [D, nkt_own * P], BF, kind="ExternalInput")
    Wq_t = nc.dram_tensor("Wq", [D, D], BF, kind="ExternalInput")
    Wk_t = nc.dram_tensor("Wk", [D, D], BF, kind="ExternalInput")
    Wv_t = nc.dram_tensor("Wv", [2 * P, ND * 512], BF, kind="ExternalInput")
    maskT_t = nc.dram_tensor("maskT", [P, 2 * P], dt.float32, kind="ExternalInput")
    ones_t = nc.dram_tensor("ones", [P, 1], BF, kind="ExternalInput")
    O_t = nc.dram_tensor("O", [SLOTS * P, D], dt.float32, kind="ExternalOutput")

    aps = (xqT_t.ap(), xT_t.ap(), Wq_t.ap(), Wk_t.ap(), Wv_t.ap(),
           maskT_t.ap(), ones_t.ap(), O_t.ap())

    with tile.TileContext(nc) as tc:
        with (
            tc.tile_pool(name="const", bufs=1) as sb_const,
            tc.tile_pool(name="xq", bufs=1) as sb_xq,
            tc.tile_pool(name="xt", bufs=1) as sb_xt,
            tc.tile_pool(name="qt", bufs=1) as sb_qt,
            tc.tile_pool(name="kt", bufs=1) as sb_kt,
            tc.tile_pool(name="vt", bufs=1) as sb_vt,
            tc.tile_pool(name="w", bufs=3) as sb_w,
            tc.tile_pool(name="wv", bufs=2) as sb_wv,
            tc.tile_pool(name="pt", bufs=1) as sb_pt,
            tc.tile_pool(name="small", bufs=2) as sb_small,
            tc.tile_pool(name="ob", bufs=4) as sb_ob,
            tc.tile_pool(name="cc", bufs=1, space=bass.MemorySpace.DRAM) as dr_cc,
            tc.tile_pool(name="ps", bufs=1, space=bass.MemorySpace.PSUM) as ps,
        ):
            pools = (sb_const, sb_xq, sb_xt, sb_qt, sb_kt, sb_vt, sb_w, sb_wv,
                     sb_pt, sb_small, sb_ob, dr_cc, ps)
            if reps == 1:
                _emit_body(nc, tc, pools, aps)
            else:
                with tc.For_i(0, reps, 1):
                    _emit_body(nc, tc, pools, aps)

    nc.compile()
    _dedup_ldweights(nc)
    return nc


def _dedup_ldweights(nc):
    """Delete an InstLdweights when the PE stream is (lw_A, mm_A, lw_B, mm_B)
    with identical weight APs and lw_B carries no sem waits/updates: mm_B then
    streams against the still-loaded weights (valid for non-fp32 stationaries).
    """
    def wkey(lw):
        a = lw.ins[0]
        return (a.memref, a.offset, str(a.ap), str(a.dtype), lw.is_transpose,
                lw.perf_mode, str(lw.tile_position), str(lw.tile_size))

    def clean(si):
        return si is None or (not si.on_wait and not si.on_update)

    removed = 0
    for blk in nc.m.functions[0].blocks:
        insts = list(blk.instructions)
        drop = set()
        loaded = None   # wkey of the weights currently in the PE array
        for j, i in enumerate(insts):
            tn = type(i).__name__
            if tn == "InstLdweights":
                k = wkey(i)
                if (loaded is not None and k == loaded and not i.is_transpose
                        and str(i.ins[0].dtype) == str(mybir.dt.bfloat16)
                        and clean(i.sync_info)):
                    drop.add(j)
                else:
                    loaded = k
            elif tn == "InstMatmult":
                if i.ldweights is not False:   # self-loading matmult clobbers
                    loaded = None
        if drop:
            blk.instructions = [i for j, i in enumerate(insts) if j not in drop]
            removed += len(drop)
    if removed:
        import os
        if os.environ.get("LW_DEBUG"):
            print(f"_dedup_ldweights: removed {removed}")
    return removed


def to_bf16(a):
    return np.ascontiguousarray(np.asarray(a, dtype=np.float32).astype(ml_dtypes.bfloat16))


def pack_w_cols(W, cw):
    """[D, D] -> [(D//cw)*P, ND*cw]: row (e*P+p), col (d*cw+c) = W[d*P+p, e*cw+c]."""
    ne = D // cw
    return np.ascontiguousarray(
        W.reshape(ND, P, ne, cw).transpose(2, 1, 0, 3).reshape(ne * P, ND * cw))


def make_in_maps(x, Wq, Wk, Wv):
    x = np.asarray(x, np.float32).reshape(B, S, D)
    Wqp = to_bf16(pack_w_cols(np.asarray(Wq, np.float32), P))
    Wkp = to_bf16(pack_w_cols(np.asarray(Wk, np.float32), P))
    Wvp = to_bf16(pack_w_cols(np.asarray(Wv, np.float32), 512))
    ones = to_bf16(np.ones((P, 1), np.float32))
    # triT[k, q] = 0 where k <= q else NEG (S^T orientation)
    triT = np.where(np.arange(P)[:, None] <= np.arange(P)[None, :],
                    0.0, NEG).astype(np.float32)
    masks = [
        np.concatenate([triT, np.full((P, P), NEG, np.float32)], axis=1),  # parity 0
        np.concatenate([np.zeros((P, P), np.float32), triT], axis=1),      # parity 1
    ]
    xT = [to_bf16(x[b].T) for b in range(B)]   # [D, S] per batch
    in_maps = []
    for c in range(NCORES):
        b, r = c // 2, c % 2
        xTb = xT[b]
        cols = np.concatenate([np.arange((2 * j + r) * P, (2 * j + r + 1) * P)
                               for j in range(SLOTS)])
        xqTb = np.ascontiguousarray(xTb[:, cols])
        xkeys = (np.ascontiguousarray(xTb[:, r * (S // 2):(r + 1) * (S // 2)])
                 if USE_CC else xTb)
        in_maps.append({
            "xqT": xqTb, "xT": xkeys, "Wq": Wqp, "Wk": Wkp, "Wv": Wvp,
            "maskT": masks[r], "ones": ones,
        })
    return in_maps


def assemble_output(results):
    out = np.empty((B, S, D), dtype=np.float32)
    for c in range(NCORES):
        b, r = c // 2, c % 2
        oc = results[c]["O"].reshape(SLOTS, P, D)
        for j in range(SLOTS):
            out[b, (2 * j + r) * P:(2 * j + r + 1) * P, :] = oc[j]
    return out


_nc_cache = {}


def _get_program(reps: int = 1):
    if reps not in _nc_cache:
        _nc_cache[reps] = build_program(reps)
    return _nc_cache[reps]


def kernel(x, Wq, Wk, Wv):
    x = np.asarray(x, dtype=np.float32)
    Wq = np.asarray(Wq, dtype=np.float32)
    Wk = np.asarray(Wk, dtype=np.float32)
    Wv = np.asarray(Wv, dtype=np.float32)
    nc = _get_program(1)
    in_maps = make_in_maps(x, Wq, Wk, Wv)
    results = run_bass_kernel_spmd(nc, in_maps, list(range(NCORES))).results
    return assemble_output(results)


# revision 6
# speedup vs baseline: 1.0228x; 1.0228x over previous
"""Causal single-head attention (B=4, S=2048, D=1024, fp32) on 8 TRN2 NeuronCores.

Sharding: core c <-> (batch c//2, parity r=c%2). Each core owns the 8 even/odd
128-row query tiles of its batch (balanced causal work).

v2 layout (vs baseline): all on-chip operands bf16 (fp32 PSUM accumulation),
scores computed TRANSPOSED (sT[k,q] = sum_e K^T[e,k]^T Q^T[e,q]) so the exp'd
probabilities feed the P^T V matmul directly as stationary blocks - no PE
transposes, no DVE copies on the critical path. Attention runs in two slot-quad
groups (slots 0-3 over k-tiles 0-7, slots 4-7 over k-tiles 0-15); for each
k-tile only the eligible suffix of slots is computed (zero wasted FLOPs).
Softmax denominators accumulate via ones-vector matmuls into a [1, 512] PSUM
row per group, transposed to per-partition scalars with tiny PE transposes.
Whole problem fits SBUF in bf16: no DRAM partial-O roundtrip.

USE_CC=True additionally halves K/V projection work: each core projects only
its parity's 1024 keys and the batch pair exchanges halves with an AllGather
over replica groups [[0,1],[2,3],[4,5],[6,7]].

Self-contained: hardcodes shapes; reads nothing from disk.
"""
import sys

import numpy as np
import ml_dtypes

try:
    from concourse import bass, bacc, tile
except ImportError:  # concourse ships with the container, not this file
    for _p in ("/opt/trn_rl_repo", "/root/.axon_site/_ro/trn_rl_repo"):
        if _p not in sys.path:
            sys.path.append(_p)
    from concourse import bass, bacc, tile
from concourse import mybir
from concourse.bass_utils import run_bass_kernel_spmd

dt = mybir.dt
AF = mybir.ActivationFunctionType

B, S, D = 4, 2048, 1024
P = 128
ND = D // P          # 8 d-tiles (contraction of projections)
NE = D // P          # 8 e-tiles (output feature tiles)
SLOTS = 8            # q-tiles per core
NT = S // P          # 16 k-tiles per batch
NCORES = 8
SCALE = 1.0 / float(np.sqrt(D))
NEG = -1.0e30
USE_CC = False
CC_GROUPS = [[0, 1], [2, 3], [4, 5], [6, 7]]
BF = dt.bfloat16


def _emit_body(nc, tc, pools, aps):
    (sb_const, sb_xq, sb_xt, sb_qt, sb_kt, sb_vt, sb_w, sb_wv, sb_pt,
     sb_small, sb_ob, dr_cc, ps) = pools
    xqT, xT, Wq, Wk, Wv, maskT, ones_in, O = aps

    # first weight block ahead of the bulk loads so the PE can start early;
    # constants go on the Activation queue (not needed until attention)
    wb0 = sb_w.tile([P, D], BF, tag="wblk", name="wb0")
    nc.sync.dma_start(wb0[:], (Wk if USE_CC else Wq)[0:P, :])
    ones_sb = sb_const.tile([P, 1], BF, tag="ones", name="ones_sb")
    nc.scalar.dma_start(ones_sb[:], ones_in[:])
    mask_sb = sb_const.tile([P, 2 * P], dt.float32, tag="mask", name="mask_sb")
    nc.scalar.dma_start(mask_sb[:], maskT[:])
    id1 = sb_const.tile([1, 1], dt.float32, tag="id1", name="id1")
    nc.vector.memset(id1[:], 1.0)

    # ---- input DMAs (query columns on SP; key columns on the idle
    # Activation HWDGE queue so DGE configs issue in parallel)
    xqt = [sb_xq.tile([P, SLOTS * P], BF, tag=f"xq{d}", name=f"xqt{d}")
           for d in range(ND)]
    nkt_own = NT // 2 if USE_CC else NT        # k-tiles this core projects
    xt = [sb_xt.tile([P, nkt_own * P], BF, tag=f"xt{d}", name=f"xt{d}")
          for d in range(ND)]
    if USE_CC:
        for d in range(ND):
            nc.sync.dma_start(xt[d][:], xT[d * P:(d + 1) * P, :])
        for d in range(ND):
            nc.scalar.dma_start(xqt[d][:], xqT[d * P:(d + 1) * P, :])
    else:
        for d in range(ND):
            nc.sync.dma_start(xqt[d][:, 0:512], xqT[d * P:(d + 1) * P, 0:512])
            nc.scalar.dma_start(xqt[d][:, 512:1024], xqT[d * P:(d + 1) * P, 512:1024])
        for d in range(ND):
            nc.scalar.dma_start(xt[d][:], xT[d * P:(d + 1) * P, :])

    qt = [sb_qt.tile([P, SLOTS * P], BF, tag=f"qt{e}", name=f"qt{e}")
          for e in range(NE)]
    kt = [sb_kt.tile([P, NT * P], BF, tag=f"kt{e}", name=f"kt{e}")
          for e in range(NE)]
    vt = [sb_vt.tile([P, D], BF, tag=f"vt{t}", name=f"vt{t}")
          for t in range(NT)]

    def q_proj(wq_first):
        for e in range(NE):
            if e == 0 and wq_first is not None:
                wb = wq_first
            else:
                wb = sb_w.tile([P, D], BF, tag="wblk", name="wb")
                nc.sync.dma_start(wb[:], Wq[e * P:(e + 1) * P, :])
            for c in range(2):
                qp = ps.tile([P, 512], dt.float32, tag="o", bufs=4, name="qp")
                for d in range(ND):
                    nc.tensor.matmul(qp[:], wb[:, d * P:(d + 1) * P],
                                     xqt[d][:, c * 512:(c + 1) * 512],
                                     start=(d == 0), stop=(d == ND - 1))
                nc.vector.tensor_copy(qt[e][:, c * 512:(c + 1) * 512], qp[:])

    def k_proj(kdst, wk_first):
        nchunks = (nkt_own * P) // 512
        for e in range(NE):
            if e == 0 and wk_first is not None:
                wb = wk_first
            else:
                wb = sb_w.tile([P, D], BF, tag="wblk", name="wb")
                nc.sync.dma_start(wb[:], Wk[e * P:(e + 1) * P, :])
            for c in range(nchunks):
                kp = ps.tile([P, 512], dt.float32, tag="o", bufs=4, name="kp")
                for d in range(ND):
                    nc.tensor.matmul(kp[:], wb[:, d * P:(d + 1) * P],
                                     xt[d][:, c * 512:(c + 1) * 512],
                                     start=(d == 0), stop=(d == ND - 1))
                nc.vector.tensor_copy(kdst[e][:, c * 512:(c + 1) * 512], kp[:])

    def v_proj(vdst):
        for ec in range(2):
            wv = sb_wv.tile([P, ND * 512], BF, tag="wv", name="wv")
            nc.scalar.dma_start(wv[:], Wv[ec * P:(ec + 1) * P, :])
            for st in range(nkt_own):
                vp = ps.tile([P, 512], dt.float32, tag="o", bufs=4, name="vp")
                for d in range(ND):
                    nc.tensor.matmul(vp[:], xt[d][:, st * P:(st + 1) * P],
                                     wv[:, d * 512:(d + 1) * 512],
                                     start=(d == 0), stop=(d == ND - 1))
                nc.vector.tensor_copy(vdst[st][:, ec * 512:(ec + 1) * 512], vp[:])

    if not USE_CC:
        q_proj(wb0)
        k_proj(kt, None)
        v_proj(vt)
    else:
        # K proj first -> AllGather K early; V proj + Q proj cover the
        # collective latency; attention scores gate only on AG-K.
        half = nkt_own * P
        kown = [sb_w.tile([P, half], BF, tag=f"ko{e}", bufs=1, name=f"kown{e}")
                for e in range(NE)]
        vown = [sb_w.tile([P, D], BF, tag=f"vo{t}", bufs=1, name=f"vown{t}")
                for t in range(nkt_own)]
        k_proj(kown, wb0)
        cck_in = dr_cc.tile([NE * P, D], BF, tag="ccki", name="cck_in")
        for e in range(NE):
            nc.sync.dma_start(cck_in[e * P:(e + 1) * P, :], kown[e][:])
        cck_out = dr_cc.tile([2 * NE * P, D], BF, tag="ccko", name="cck_out")
        nc.gpsimd.collective_compute(
            "AllGather", mybir.AluOpType.bypass, CC_GROUPS,
            ins=[cck_in[:]], outs=[cck_out[:]])
        v_proj(vown)
        ccv_in = dr_cc.tile([nkt_own * P, D], BF, tag="ccvi", name="ccv_in")
        for st in range(nkt_own):
            nc.sync.dma_start(ccv_in[st * P:(st + 1) * P, :], vown[st][:])
        ccv_out = dr_cc.tile([2 * nkt_own * P, D], BF, tag="ccvo", name="ccv_out")
        nc.gpsimd.collective_compute(
            "AllGather", mybir.AluOpType.bypass, CC_GROUPS,
            ins=[ccv_in[:]], outs=[ccv_out[:]])
        for e in range(NE):
            nc.sync.dma_start(kt[e][:, 0:half], cck_out[e * P:(e + 1) * P, :])
            nc.sync.dma_start(kt[e][:, half:NT * P],
                              cck_out[(NE + e) * P:(NE + e + 1) * P, :])
        q_proj(None)
        for t in range(NT):
            r_src, st = t // nkt_own, t % nkt_own
            nc.sync.dma_start(
                vt[t][:],
                ccv_out[(r_src * nkt_own + st) * P:(r_src * nkt_own + st + 1) * P, :])

    # ---- attention in two slot-quad groups
    # group g: slots jb..jb+3 (jb = 4g), k-tiles 0..8(g+1)-1.
    # For k-tile t only the eligible slot suffix jmin(t)=max(jb, t//2)..jb+3 is
    # computed; the first block of the suffix is the diagonal slot and gets the
    # parity mask block (t mod 2) when t//2 >= jb.
    for g in range(2):
        jb = 4 * g
        ntile = 8 * (g + 1)
        # scores + exp + denominator accumulation
        pts = {}
        den_ps = ps.tile([1, 512], dt.float32, tag="den", bufs=1, name="den_ps")
        for t in range(ntile):
            jmin = max(jb, t // 2)
            w = (jb + 4 - jmin) * P
            sps = ps.tile([P, w], dt.float32, tag="s", bufs=3, name="sps")
            for e in range(NE):
                nc.tensor.matmul(sps[:], kt[e][:, t * P:(t + 1) * P],
                                 qt[e][:, jmin * P:(jb + 4) * P],
                                 start=(e == 0), stop=(e == NE - 1))
            if t // 2 >= jb:
                mb = t % 2
                nc.vector.tensor_add(sps[:, 0:P], sps[:, 0:P],
                                     mask_sb[:, mb * P:(mb + 1) * P])
            ptile = sb_pt.tile([P, w], BF, tag=f"pt{t}", name=f"pt{t}")
            nc.scalar.activation(ptile[:], sps[:], AF.Exp, scale=SCALE)
            pts[t] = (ptile, jmin)
            if t > 0:
                pm, pjmin = pts[t - 1]
                nc.tensor.matmul(den_ps[0:1, (pjmin - jb) * P:512], ones_sb[:],
                                 pm[:], start=(t - 1 == 0), stop=False)
            if t == ntile - 1:
                off = (jmin - jb) * P
                nc.tensor.matmul(den_ps[0:1, off:512], ones_sb[:], ptile[:],
                                 start=False, stop=True)
        # denominator -> per-partition reciprocals
        den_sb = sb_small.tile([1, 512], dt.float32, tag="den_sb", name="den_sb")
        nc.vector.tensor_copy(den_sb[:], den_ps[0:1, :])
        rcp_sb = sb_small.tile([1, 512], dt.float32, tag="rcp_sb", name="rcp_sb")
        nc.vector.reciprocal(rcp_sb[:], den_sb[:])
        rec_ps = ps.tile([P, 4], dt.float32, tag="den", bufs=1, name="rec_ps")
        for jj in range(4):
            nc.tensor.transpose(rec_ps[:, jj:jj + 1],
                                rcp_sb[0:1, jj * P:(jj + 1) * P],
                                id1[:])
        rec_sb = sb_small.tile([P, 4], dt.float32, tag="rec_sb", name="rec_sb")
        nc.vector.tensor_copy(rec_sb[:], rec_ps[:])

        # P^T V with per-slot PSUM accumulation, then normalize + store
        for j in range(jb, jb + 4):
            next_t = 2 * j + 2
            for h in range(2):
                ops = ps.tile([P, 512], dt.float32, tag="o", bufs=4, name="ops")
                for t in range(next_t):
                    ptile, jmin = pts[t]
                    cb = (j - jmin) * P
                    nc.tensor.matmul(ops[:], ptile[:, cb:cb + P],
                                     vt[t][:, h * 512:(h + 1) * 512],
                                     start=(t == 0), stop=(t == next_t - 1))
                ob = sb_ob.tile([P, 512], dt.float32, tag="ob", name="ob")
                nc.scalar.mul(ob[:], ops[:], rec_sb[:, j - jb:j - jb + 1])
                nc.sync.dma_start(O[j * P:(j + 1) * P, h * 512:(h + 1) * 512], ob[:])


def build_program(reps: int = 1):
    nc = bacc.Bacc("TRN2", target_bir_lowering=False, debug=False,
                   num_devices=NCORES)

    nkt_own = NT // 2 if USE_CC else NT
    xqT_t = nc.dram_tensor("xqT", [D, SLOTS * P], BF, kind="ExternalInput")
    xT_t = nc.dram_tensor("xT", [D, nkt_own * P], BF, kind="ExternalInput")
    Wq_t = nc.dram_tensor("Wq", [D, D], BF, kind="ExternalInput")
    Wk_t = nc.dram_tensor("Wk", [D, D], BF, kind="ExternalInput")
    Wv_t = nc.dram_tensor("Wv", [2 * P, ND * 512], BF, kind="ExternalInput")
    maskT_t = nc.dram_tensor("maskT", [P, 2 * P], dt.float32, kind="ExternalInput")
    ones_t = nc.dram_tensor("ones", [P, 1], BF, kind="ExternalInput")
    O_t = nc.dram_tensor("O", [SLOTS * P, D], dt.float32, kind="ExternalOutput")

    aps = (xqT_t.ap(), xT_t.ap(), Wq_t.ap(), Wk_t.ap(), Wv_t.ap(),
           maskT_t.ap(), ones_t.ap(), O_t.ap())

    with tile.TileContext(nc) as tc:
        with (
            tc.tile_pool(name="const", bufs=1) as sb_const,
            tc.tile_pool(name="xq", bufs=1) as sb_xq,
            tc.tile_pool(name="xt", bufs=1) as sb_xt,
            tc.tile_pool(name="qt", bufs=1) as sb_qt,
            tc.tile_pool(name="kt", bufs=1) as sb_kt,
            tc.tile_pool(name="vt", bufs=1) as sb_vt,
            tc.tile_pool(name="w", bufs=3) as sb_w,
            tc.tile_pool(name="wv", bufs=2) as sb_wv,
            tc.tile_pool(name="pt", bufs=1) as sb_pt,
            tc.tile_pool(name="small", bufs=2) as sb_small,
            tc.tile_pool(name="ob", bufs=4) as sb_ob,
            tc.tile_pool(name="cc", bufs=1, space=bass.MemorySpace.DRAM) as dr_cc,
            tc.tile_pool(name="ps", bufs=1, space=bass.MemorySpace.PSUM) as ps,
        ):
            pools = (sb_const, sb_xq, sb_xt, sb_qt, sb_kt, sb_vt, sb_w, sb_wv,
                     sb_pt, sb_small, sb_ob, dr_cc, ps)
            if reps == 1:
                _emit_body(nc, tc, pools, aps)
            else:
                with tc.For_i(0, reps, 1):
                    _emit_body(nc, tc, pools, aps)

    nc.compile()
    return nc


def to_bf16(a):
    return np.ascontiguousarray(np.asarray(a, dtype=np.float32).astype(ml_dtypes.bfloat16))


def pack_w_cols(W, cw):
    """[D, D] -> [(D//cw)*P, ND*cw]: row (e*P+p), col (d*cw+c) = W[d*P+p, e*cw+c]."""
    ne = D // cw
    return np.ascontiguousarray(
        W.reshape(ND, P, ne, cw).transpose(2, 1, 0, 3).reshape(ne * P, ND * cw))


def make_in_maps(x, Wq, Wk, Wv):
    x = np.asarray(x, np.float32).reshape(B, S, D)
    Wqp = to_bf16(pack_w_cols(np.asarray(Wq, np.float32), P))
    Wkp = to_bf16(pack_w_cols(np.asarray(Wk, np.float32), P))
    Wvp = to_bf16(pack_w_cols(np.asarray(Wv, np.float32), 512))
    ones = to_bf16(np.ones((P, 1), np.float32))
    # triT[k, q] = 0 where k <= q else NEG (S^T orientation)
    triT = np.where(np.arange(P)[:, None] <= np.arange(P)[None, :],
                    0.0, NEG).astype(np.float32)
    masks = [
        np.concatenate([triT, np.full((P, P), NEG, np.float32)], axis=1),  # parity 0
        np.concatenate([np.zeros((P, P), np.float32), triT], axis=1),      # parity 1
    ]
    xT = [to_bf16(x[b].T) for b in range(B)]   # [D, S] per batch
    in_maps = []
    for c in range(NCORES):
        b, r = c // 2, c % 2
        xTb = xT[b]
        cols = np.concatenate([np.arange((2 * j + r) * P, (2 * j + r + 1) * P)
                               for j in range(SLOTS)])
        xqTb = np.ascontiguousarray(xTb[:, cols])
        xkeys = (np.ascontiguousarray(xTb[:, r * (S // 2):(r + 1) * (S // 2)])
                 if USE_CC else xTb)
        in_maps.append({
            "xqT": xqTb, "xT": xkeys, "Wq": Wqp, "Wk": Wkp, "Wv": Wvp,
            "maskT": masks[r], "ones": ones,
        })
    return in_maps


def assemble_output(results):
    out = np.empty((B, S, D), dtype=np.float32)
    for c in range(NCORES):
        b, r = c // 2, c % 2
        oc = results[c]["O"].reshape(SLOTS, P, D)
        for j in range(SLOTS):
            out[b, (2 * j + r) * P:(2 * j + r + 1) * P, :] = oc[j]
    return out


_nc_cache = {}


def _get_program(reps: int = 1):
    if reps not in _nc_cache:
        _nc_cache[reps] = build_program(reps)
    return _nc_cache[reps]


def kernel(x, Wq, Wk, Wv):
    x = np.asarray(x, dtype=np.float32)
    Wq = np.asarray(Wq, dtype=np.float32)
    Wk = np.asarray(Wk, dtype=np.float32)
    Wv = np.asarray(Wv, dtype=np.float32)
    nc = _get_program(1)
    in_maps = make_in_maps(x, Wq, Wk, Wv)
    results = run_bass_kernel_spmd(nc, in_maps, list(range(NCORES))).results
    return assemble_output(results)
